# revision 51
# baseline (speedup 1.0000x reference)
"""Trainium2 Bass kernel for nn_Attention_7078106104284.

Self-attention block (SAGAN-style) over x[8, 256, 64, 64]:
  q = wq@x+bq [32,n], k = wk@x+bk [32,n], v = wv@x+bv [256,n], n = 4096
  attn = softmax(q^T k, axis=m);  y = x + gamma * (v @ attn^T)

Sharding: data-parallel over batch - one batch element per NeuronCore (8 cores).

Numerics: plain bf16 matmuls throughout (fp32 PSUM accumulation). Measured on
the actual task data, logit-path hi/lo splits change the final error not at
all - the bf16 output/residual quantization (~0.6% of out-scale, vs the 2%
gate) dominates. Bias algebra:
  - bk drops entirely (q.bk is constant per softmax row -> cancels),
  - bq fuses into the q evacuation on ACT,
  - bv folds into the residual (sum_m attn = 1), precomputed on host.
Softmax max-subtraction skipped (|logit| < 50 << 88; exp and Z ride in
f32/bf16 range).

Dataflow per core (DMA count kept low - each dma_start costs ~0.6-2us of
serial ring time; loop-allocated tiles get distinct pool tags so their DMAs
are not serialized behind the previous tile's consumers):
  - x (bf16) arrives in 4 chunks (1+2+2+3 n-groups), each split across both
    HWDGE rings (SP + ACT); projections start on chunk 0 immediately.
  - q+k share one stationary [128, 64]: each chunk-group needs just TWO
    accumulating matmuls into pqk[64, 512] (q rows 0:32, k rows 32:64).
    ACT evacuates q (bias fused) into q_rep[0:32] and k into k_rep[32:64];
    one partition-shift DMA per chunk completes each replica pair.
  - vT[m, c'] via stationary x chunks, interleaved with projections; the
    Z ones-column is one strided memset; copies on DVE.
  - attention, one group deep in software pipeline: per round TWO logit
    matmuls (K=32) issued at row_grp 0 and 32 via base-partition-derived
    tile_position - the PE array runs them CONCURRENTLY in different 32-row
    strips -> one fused exp on ACT over [128, 2*512] bf16 -> 8 AV matmuls
    accumulating uT[n-sub, c'] (c'=256 ones column carries Z).
  - epilogue per sub: DVE normalize (gamma/Z), GPSIMD adds the residual
    (x^T + gamma*bv, bf16), one merged y DMA per group in [N, C] layout
    (host transposes back). No PE transposes anywhere.
"""

import sys

sys.path.insert(0, "/opt/trn_rl_repo")

import numpy as np
from contextlib import ExitStack

import concourse.bass as bass
import concourse.bacc as bacc
import concourse.tile as tile
import concourse.mybir as mybir
from concourse.bass_utils import run_bass_kernel_spmd

dt = mybir.dt
AF = mybir.ActivationFunctionType

B = 8
C = 256
C8 = 32
N = 4096          # h*w spatial positions
NG = 512          # n-group width (one PSUM bank of fp32)
G = N // NG       # 8 n-groups
MC = N // 128     # 32 m-chunks
EW = 2            # m-chunks per exp batch (PSUM banks per plt buffer)
RND = MC // EW    # 16 rounds per group
CP = C + 1        # AV output channels incl. the Z ones-column


def build_program(reps=1, ablate=()):
    nc = bacc.Bacc("TRN2", target_bir_lowering=False)
    f32 = dt.float32
    bf16 = dt.bfloat16
    xh_d = nc.declare_dram_parameter("x_h", [C, N], bf16, isOutput=False)
    xr_d = nc.declare_dram_parameter("x_res", [N, C], bf16, isOutput=False)
    # all bf16 weights in ONE tensor, partition-major:
    #   [:, 0:128] = wqkT cc-slots (64 each), [:, 128:640] = wvT (cc, 256)
    wpk_d = nc.declare_dram_parameter("wpack", [128, 640], bf16, isOutput=False)
    # col 0 = gamma replicated x128 by host, col 1 rows 0:32 = bq
    bqg_d = nc.declare_dram_parameter("bqg", [128, 2], f32, isOutput=False)
    y_d = nc.declare_dram_parameter("y", [N, C], bf16, isOutput=True)

    with tile.TileContext(nc) as tc, ExitStack() as ctx:
        sing = ctx.enter_context(tc.tile_pool(name="sing", bufs=1))
        xpool = ctx.enter_context(tc.tile_pool(name="xpool", bufs=1))
        # e tiles live a full group (RND rounds) before AV consumes them
        epool = ctx.enter_context(tc.tile_pool(name="epool", bufs=RND + 2))
        ypool = ctx.enter_context(tc.tile_pool(name="ypool", bufs=2))
        scal = ctx.enter_context(tc.tile_pool(name="scal", bufs=4))

        lt_ps = ctx.enter_context(tc.tile_pool(name="lt_ps", bufs=2, space="PSUM"))
        u_ps = ctx.enter_context(tc.tile_pool(name="u_ps", bufs=1, space="PSUM"))

        for _rep in range(reps):
            xh_view = xh_d[:].rearrange("(cc p) m -> p cc m", p=128)
            # x chunks (in n-groups): small first chunk so projections start
            # early; halves split across both HWDGE rings (SP + ACT).
            CHUNK_GROUPS = [[0], [1, 2], [3, 4], [5, 6, 7]]
            grp_chunk = {}
            grp_off = {}
            for ci, gs in enumerate(CHUNK_GROUPS):
                for oi, g_ in enumerate(gs):
                    grp_chunk[g_] = ci
                    grp_off[g_] = oi
            xh_t = []
            for cidx, gs in enumerate(CHUNK_GROUPS):
                cw = len(gs) * NG
                c0 = gs[0] * NG
                t = xpool.tile([128, 2, cw], bf16, tag=f"x{cidx}", name=f"x{cidx}")
                nc.sync.dma_start(out=t[:, 0:1, :], in_=xh_view[:, 0:1, c0:c0 + cw])
                nc.scalar.dma_start(out=t[:, 1:2, :], in_=xh_view[:, 1:2, c0:c0 + cw])
                xh_t.append(t)
                if cidx == 0:
                    wpk_sb = sing.tile([128, 640], bf16)
                    nc.sync.dma_start(out=wpk_sb, in_=wpk_d[:])
                    bqg_sb = sing.tile([128, 2], f32)
                    nc.scalar.dma_start(out=bqg_sb, in_=bqg_d[:])
                    bq_sb = bqg_sb[0:C8, 1:2]
                    g128 = bqg_sb[:, 0:1]

            def wqk_slot(cc):
                return wpk_sb[:, cc * 64:(cc + 1) * 64]

            def wv_slot(cc):
                return wpk_sb[:, 128 + cc * C:128 + (cc + 1) * C]

            # ---- q/k projections + vT, per group ----
            # q_rep/k_rep hold q (k) on ALL FOUR partition strips so logit
            # matmuls can be issued row-tiled at row_grp 0/32/64/96 - the PE
            # array runs four K=32 matmuls concurrently.
            q_rep = sing.tile([128, N], bf16)
            k_rep = sing.tile([128, N], bf16)
            vt_sb = sing.tile([128, MC, CP], bf16)
            nc.vector.memset(vt_sb[:, :, C:CP], 1.0)   # Z ones-column
            for s in range(G):
                sl = slice(s * NG, (s + 1) * NG)
                csl = slice(grp_off[s] * NG, (grp_off[s] + 1) * NG)
                xc = xh_t[grp_chunk[s]]
                pqk = u_ps.tile([64, NG], f32, tag=f"u{s % 2}", name="pqk")
                for cc in range(2):
                    nc.tensor.matmul(pqk, wqk_slot(cc), xc[:, cc, csl],
                                     start=(cc == 0), stop=(cc == 1))
                # ACT evacuation: q with fused bias, k plain (lane-aligned)
                nc.scalar.activation(q_rep[0:C8, sl], pqk[0:C8, :],
                                     AF.Identity, bias=bq_sb)
                nc.scalar.activation(k_rep[C8:64, sl], pqk[C8:64, :], AF.Copy)

                # vT for this group's 4 m-chunks (copies on DVE)
                for mc in range(4 * s, 4 * s + 4):
                    msl = slice((grp_off[s] * 4 + mc % 4) * 128,
                                (grp_off[s] * 4 + mc % 4 + 1) * 128)
                    pv = u_ps.tile([128, C], f32, tag=f"u{2 + mc % 2}", name="pv")
                    for cc in range(2):
                        nc.tensor.matmul(pv, xc[:, cc, msl], wv_slot(cc),
                                         start=(cc == 0), stop=(cc == 1))
                    nc.vector.tensor_copy(vt_sb[:, mc, 0:C], pv)

                # replica completion per finished chunk (q on ACT ring, k on
                # SP ring): one replicating partition-shift DMA each fans the
                # natural strip out to the other three
                if s == CHUNK_GROUPS[grp_chunk[s]][-1]:
                    gs = CHUNK_GROUPS[grp_chunk[s]]
                    dsl = slice(gs[0] * NG, (gs[-1] + 1) * NG)
                    # all replicas read the natural strip: no serial DMA
                    # chain on the projections->attention critical path
                    for r in (C8, 64, 96):
                        nc.scalar.dma_start(out=q_rep[r:r + C8, dsl],
                                            in_=q_rep[0:C8, dsl])
                    for r in (0, 64, 96):
                        nc.sync.dma_start(out=k_rep[r:r + C8, dsl],
                                          in_=k_rep[C8:64, dsl])

            # residual chunks (first needed ~20us into attention)
            xr_t = []
            for h in range(2):
                tr = sing.tile([128, 16, C], bf16, tag=f"xr{h}", name=f"xr{h}")
                nc.sync.dma_start(
                    out=tr,
                    in_=xr_d[:].rearrange("(gs p) c -> p gs c", p=128)[
                        :, h * 16:(h + 1) * 16, :])
                xr_t.append(tr)

            # ---- attention, software-pipelined one group deep ----
            e_tiles = {}
            u_tiles = {}

            def issue_lt_exp(g, j):
                # rounds are emitted in pairs: even j uses row_grps 0/32,
                # odd j 64/96, so four K=32 logit matmuls sit back-to-back in
                # the PE queue and run concurrently in the four 32-row strips
                sl = slice(g * NG, (g + 1) * NG)
                plt = lt_ps.tile([128, EW, NG], f32, tag="plt", name="plt")
                for rg in range(EW if "lt" not in ablate else 1):
                    mc = EW * j + rg
                    msl = slice(mc * 128, (mc + 1) * 128)
                    row = ((j % 2) * 2 + rg) * C8
                    tp = (96, 0) if row == 96 else None
                    nc.tensor.matmul(plt[:, rg, :], k_rep[row:row + C8, msl],
                                     q_rep[row:row + C8, sl],
                                     start=True, stop=True, tile_position=tp)
                e_t = epool.tile([128, EW, NG], bf16, tag="e", name="e_t")
                fn = AF.Exp if "exp" not in ablate else AF.Copy
                nc.scalar.activation(e_t, plt, fn)
                e_tiles[(g, j)] = e_t

            def issue_av(g, j):
                uts = u_tiles[g]
                e_t = e_tiles.pop((g, j))
                if "av" in ablate:
                    if j == 0:
                        for sub in range(4):
                            nc.tensor.matmul(uts[sub],
                                             e_t[:, 0, sub * 128:(sub + 1) * 128],
                                             vt_sb[:, 0, :], start=True, stop=True)
                    return
                if j == RND - 1:
                    # last round sub-major: each sub's accumulation stops as
                    # early as possible so its epilogue overlaps remaining AV
                    for sub in range(4):
                        for rg in range(EW):
                            mc = EW * j + rg
                            nc.tensor.matmul(uts[sub],
                                             e_t[:, rg, sub * 128:(sub + 1) * 128],
                                             vt_sb[:, mc, :],
                                             start=False, stop=(rg == EW - 1))
                    return
                for rg in range(EW):
                    mc = EW * j + rg
                    first = (j == 0 and rg == 0)
                    for sub in range(4):
                        nc.tensor.matmul(uts[sub],
                                         e_t[:, rg, sub * 128:(sub + 1) * 128],
                                         vt_sb[:, mc, :],
                                         start=first, stop=False)

            def issue_epilogue(g):
                # DVE normalizes (gamma/Z); GPSIMD adds the bf16 residual.
                # Last group streams each sub's y out separately (short tail).
                uts = u_tiles.pop(g)
                y_view = y_d[:].rearrange("(gs p) c -> p gs c", p=128)
                y_g = ypool.tile([128, 4, C], bf16, tag="yg", name="y_g")
                for sub in range(4):
                    ut = uts[sub]
                    rinv = scal.tile([128, 1], f32, tag="rinv", name="rinv")
                    nc.vector.reciprocal(rinv, ut[:, C:CP])
                    gsc = scal.tile([128, 1], f32, tag="gsc", name="gsc")
                    nc.vector.tensor_scalar_mul(gsc, rinv, g128)
                    ysc = scal.tile([128, C], f32, tag="ysc", name="ysc")
                    nc.vector.tensor_scalar_mul(ysc, ut[:, 0:C], gsc)
                    nc.gpsimd.tensor_add(y_g[:, sub, :], ysc,
                                         xr_t[g // 4][:, (g % 4) * 4 + sub, :])
                    if g == G - 1:
                        nc.sync.dma_start(out=y_view[:, g * 4 + sub, :],
                                          in_=y_g[:, sub, :])
                if g < G - 1:
                    nc.sync.dma_start(out=y_view[:, g * 4:(g + 1) * 4, :],
                                      in_=y_g)

            # flat round-pair stream; AV lags logits by LAGP pairs. LAGP=4
            # keeps fill/drain phases short while giving each epilogue two
            # pairs (~4.6us) to drain the u banks before AV(g+1) rewrites
            # them (LAGP=2 stalled on the DVE normalize chain there).
            NP = G * (RND // 2)
            LAGP = 4
            for ip in range(NP + LAGP):
                if ip < NP:
                    g, jp = divmod(ip, RND // 2)
                    if jp == 0:
                        u_tiles[g] = [u_ps.tile([128, CP], f32, tag=f"u{s}",
                                                name=f"u{s}")
                                      for s in range(4)]
                    issue_lt_exp(g, 2 * jp)
                    issue_lt_exp(g, 2 * jp + 1)
                if ip >= LAGP:
                    ga, jpa = divmod(ip - LAGP, RND // 2)
                    issue_av(ga, 2 * jpa)
                    issue_av(ga, 2 * jpa + 1)
                    if jpa == RND // 2 - 1:
                        issue_epilogue(ga)

    nc.compile()
    return nc


def prepare_in_maps(inputs):
    """Host-side prep: bf16 casts, packed weights, residual fold."""
    import ml_dtypes
    bf = ml_dtypes.bfloat16
    x = np.asarray(inputs["x"], dtype=np.float32)
    wq = np.asarray(inputs["wq"], dtype=np.float32)
    bq = np.asarray(inputs["bq"], dtype=np.float32)
    wk = np.asarray(inputs["wk"], dtype=np.float32)
    wv = np.asarray(inputs["wv"], dtype=np.float32)
    bv = np.asarray(inputs["bv"], dtype=np.float32)
    gamma = np.asarray(inputs["gamma"], dtype=np.float32)

    xr = np.ascontiguousarray(x.reshape(B, C, N))
    x_h = xr.astype(bf)
    # residual in [N, C] layout with gamma*bv folded in (bf16: |err| ~0.4%)
    xres = np.ascontiguousarray(
        xr.transpose(0, 2, 1) + gamma[0] * bv[None, None, :]).astype(bf)

    # pack all bf16 weights partition-major into [128, 640]:
    #   cols 0:128 = 2 cc-slots of wqkT ([wq.T wk.T]), cols 128:640 = wvT
    wqkT = np.concatenate([wq.T, wk.T], axis=1).astype(bf)   # [(cc p), 64]
    wqk_pm = wqkT.reshape(2, 128, 64).transpose(1, 0, 2).reshape(128, 128)
    wvT = wv.T.astype(bf)                                    # [(cc p), C]
    wv_pm = wvT.reshape(2, 128, C).transpose(1, 0, 2).reshape(128, 512)
    wpack = np.ascontiguousarray(np.concatenate([wqk_pm, wv_pm], axis=1))
    # bqg: col 0 = gamma replicated, col 1 rows 0:32 = bq
    bqg = np.zeros((128, 2), dtype=np.float32)
    bqg[:, 0] = gamma[0]
    bqg[0:C8, 1] = bq

    shared = {"wpack": wpack, "bqg": bqg}
    return [dict(shared,
                 x_h=np.ascontiguousarray(x_h[i]),
                 x_res=xres[i]) for i in range(B)]


_nc_cache = None


def kernel(**inputs) -> np.ndarray:
    global _nc_cache
    if _nc_cache is None:
        _nc_cache = build_program()
    nc = _nc_cache

    in_maps = prepare_in_maps(inputs)
    res = run_bass_kernel_spmd(nc, in_maps, core_ids=list(range(B)))
    # y comes back [N, C] bf16 per core; transpose to [C, N] on host
    y = np.stack([res.results[i]["y"].astype(np.float32).T for i in range(B)],
                 axis=0)
    return np.ascontiguousarray(y.reshape(B, C, 64, 64))


if __name__ == "__main__":
    rng = np.random.default_rng(0)
    ins = {
        "x": rng.standard_normal((B, C, 64, 64), dtype=np.float32),
        "wq": rng.standard_normal((C8, C), dtype=np.float32) / 16,
        "bq": rng.standard_normal((C8,), dtype=np.float32) * 0.01,
        "wk": rng.standard_normal((C8, C), dtype=np.float32) / 16,
        "bk": rng.standard_normal((C8,), dtype=np.float32) * 0.01,
        "wv": rng.standard_normal((C, C), dtype=np.float32) / 16,
        "bv": rng.standard_normal((C,), dtype=np.float32) * 0.01,
        "gamma": rng.standard_normal((1,), dtype=np.float32) * 0.1,
    }
    out = kernel(**ins)
    print("kernel output", out.shape, out.dtype)


# revision 53
# speedup vs baseline: 1.0603x; 1.0603x over previous
"""Trainium2 Bass kernel for nn_Attention_7078106104284.

Self-attention block (SAGAN-style) over x[8, 256, 64, 64]:
  q = wq@x+bq [32,n], k = wk@x+bk [32,n], v = wv@x+bv [256,n], n = 4096
  attn = softmax(q^T k, axis=m);  y = x + gamma * (v @ attn^T)

Sharding: data-parallel over batch - one batch element per NeuronCore (8 cores).

Numerics: plain bf16 matmuls throughout (fp32 PSUM accumulation). Measured on
the actual task data, logit-path hi/lo splits change the final error not at
all - the bf16 output/residual quantization (~0.6% of out-scale, vs the 2%
gate) dominates. Bias algebra:
  - bk drops entirely (q.bk is constant per softmax row -> cancels),
  - bq fuses into the q evacuation on ACT,
  - bv folds into the residual (sum_m attn = 1), precomputed on host.
Softmax max-subtraction skipped (|logit| < 50 << 88; exp and Z ride in
f32/bf16 range).

Dataflow per core (DMA count kept low - each dma_start costs ~0.6-2us of
serial ring time; loop-allocated tiles get distinct pool tags so their DMAs
are not serialized behind the previous tile's consumers):
  - x (bf16) arrives in 4 chunks (1+2+2+3 n-groups), each split across both
    HWDGE rings (SP + ACT); projections start on chunk 0 immediately.
  - q+k share one stationary [128, 64]: each chunk-group needs just TWO
    accumulating matmuls into pqk[64, 512] (q rows 0:32, k rows 32:64).
    ACT evacuates q (bias fused) into q_rep[0:32] and k into k_rep[32:64];
    one partition-shift DMA per chunk completes each replica pair.
  - vT[m, c'] via stationary x chunks, interleaved with projections; the
    Z ones-column is one strided memset; copies on DVE.
  - attention, one group deep in software pipeline: per round TWO logit
    matmuls (K=32) issued at row_grp 0 and 32 via base-partition-derived
    tile_position - the PE array runs them CONCURRENTLY in different 32-row
    strips -> one fused exp on ACT over [128, 2*512] bf16 -> 8 AV matmuls
    accumulating uT[n-sub, c'] (c'=256 ones column carries Z).
  - epilogue per sub: DVE normalize (gamma/Z), GPSIMD adds the residual
    (x^T + gamma*bv, bf16), one merged y DMA per group in [N, C] layout
    (host transposes back). No PE transposes anywhere.
"""

import sys

sys.path.insert(0, "/opt/trn_rl_repo")

import numpy as np
from contextlib import ExitStack

import concourse.bass as bass
import concourse.bacc as bacc
import concourse.tile as tile
import concourse.mybir as mybir
from concourse.bass_utils import run_bass_kernel_spmd

dt = mybir.dt
AF = mybir.ActivationFunctionType

B = 8
C = 256
C8 = 32
N = 4096          # h*w spatial positions
NG = 512          # n-group width (one PSUM bank of fp32)
G = N // NG       # 8 n-groups
MC = N // 128     # 32 m-chunks
EW = 2            # m-chunks per exp batch (PSUM banks per plt buffer)
RND = MC // EW    # 16 rounds per group
CP = C + 1        # AV output channels incl. the Z ones-column


def build_program(reps=1, ablate=()):
    nc = bacc.Bacc("TRN2", target_bir_lowering=False)
    f32 = dt.float32
    bf16 = dt.bfloat16
    xh_d = nc.declare_dram_parameter("x_h", [C, N], bf16, isOutput=False)
    xr_d = nc.declare_dram_parameter("x_res", [N, C], bf16, isOutput=False)
    # all bf16 weights in ONE tensor, partition-major:
    #   [:, 0:128] = wqkT cc-slots (64 each), [:, 128:640] = wvT (cc, 256)
    wpk_d = nc.declare_dram_parameter("wpack", [128, 640], bf16, isOutput=False)
    # col 0 = gamma replicated x128 by host, col 1 rows 0:32 = bq
    bqg_d = nc.declare_dram_parameter("bqg", [128, 2], f32, isOutput=False)
    y_d = nc.declare_dram_parameter("y", [N, C], bf16, isOutput=True)

    with tile.TileContext(nc) as tc, ExitStack() as ctx:
        sing = ctx.enter_context(tc.tile_pool(name="sing", bufs=1))
        xpool = ctx.enter_context(tc.tile_pool(name="xpool", bufs=1))
        # e tiles live a full group (RND rounds) before AV consumes them
        epool = ctx.enter_context(tc.tile_pool(name="epool", bufs=RND + 2))
        ypool = ctx.enter_context(tc.tile_pool(name="ypool", bufs=2))
        scal = ctx.enter_context(tc.tile_pool(name="scal", bufs=4))

        lt_ps = ctx.enter_context(tc.tile_pool(name="lt_ps", bufs=2, space="PSUM"))
        u_ps = ctx.enter_context(tc.tile_pool(name="u_ps", bufs=1, space="PSUM"))

        for _rep in range(reps):
            xh_view = xh_d[:].rearrange("(cc p) m -> p cc m", p=128)
            # x chunks (in n-groups): small first chunk so projections start
            # early; halves split across both HWDGE rings (SP + ACT).
            CHUNK_GROUPS = [[0], [1, 2], [3, 4], [5, 6, 7]]
            grp_chunk = {}
            grp_off = {}
            for ci, gs in enumerate(CHUNK_GROUPS):
                for oi, g_ in enumerate(gs):
                    grp_chunk[g_] = ci
                    grp_off[g_] = oi
            xh_t = []
            for cidx, gs in enumerate(CHUNK_GROUPS):
                cw = len(gs) * NG
                c0 = gs[0] * NG
                t = xpool.tile([128, 2, cw], bf16, tag=f"x{cidx}", name=f"x{cidx}")
                nc.sync.dma_start(out=t[:, 0:1, :], in_=xh_view[:, 0:1, c0:c0 + cw])
                nc.scalar.dma_start(out=t[:, 1:2, :], in_=xh_view[:, 1:2, c0:c0 + cw])
                xh_t.append(t)
                if cidx == 0:
                    wpk_sb = sing.tile([128, 640], bf16)
                    nc.sync.dma_start(out=wpk_sb, in_=wpk_d[:])
                    bqg_sb = sing.tile([128, 2], f32)
                    nc.sync.dma_start(out=bqg_sb, in_=bqg_d[:])
                    bq_sb = bqg_sb[0:C8, 1:2]
                    g128 = bqg_sb[:, 0:1]

            def wqk_slot(cc):
                return wpk_sb[:, cc * 64:(cc + 1) * 64]

            def wv_slot(cc):
                return wpk_sb[:, 128 + cc * C:128 + (cc + 1) * C]

            # ---- q/k projections + vT, per group ----
            # q_rep/k_rep hold q (k) on ALL FOUR partition strips so logit
            # matmuls can be issued row-tiled at row_grp 0/32/64/96 - the PE
            # array runs four K=32 matmuls concurrently.
            q_rep = sing.tile([128, N], bf16)
            k_rep = sing.tile([128, N], bf16)
            # rows padded to 264 (528B = 16B-aligned stride) so every AV
            # moving stream starts SBUF-fetch-aligned; cols 0:256 = vT,
            # col 256 = Z ones-column, 257:264 unused pad
            vt_sb = sing.tile([128, MC, 264], bf16)
            nc.vector.memset(vt_sb[:, :, C:CP], 1.0)   # Z ones-column
            for s in range(G):
                sl = slice(s * NG, (s + 1) * NG)
                csl = slice(grp_off[s] * NG, (grp_off[s] + 1) * NG)
                xc = xh_t[grp_chunk[s]]
                pqk = u_ps.tile([64, NG], f32, tag=f"u{s % 2}", name="pqk")
                for cc in range(2):
                    nc.tensor.matmul(pqk, wqk_slot(cc), xc[:, cc, csl],
                                     start=(cc == 0), stop=(cc == 1))
                # ACT evacuation: q with fused bias, k plain (lane-aligned)
                nc.scalar.activation(q_rep[0:C8, sl], pqk[0:C8, :],
                                     AF.Identity, bias=bq_sb)
                nc.scalar.activation(k_rep[C8:64, sl], pqk[C8:64, :], AF.Copy)

                # vT for this group's 4 m-chunks (copies on DVE)
                for mc in range(4 * s, 4 * s + 4):
                    msl = slice((grp_off[s] * 4 + mc % 4) * 128,
                                (grp_off[s] * 4 + mc % 4 + 1) * 128)
                    pv = u_ps.tile([128, C], f32, tag=f"u{2 + mc % 2}", name="pv")
                    for cc in range(2):
                        nc.tensor.matmul(pv, xc[:, cc, msl], wv_slot(cc),
                                         start=(cc == 0), stop=(cc == 1))
                    nc.vector.tensor_copy(vt_sb[:, mc, 0:C], pv)

                # replica completion per finished chunk (q on ACT ring, k on
                # SP ring): one replicating partition-shift DMA each fans the
                # natural strip out to the other three
                if s == CHUNK_GROUPS[grp_chunk[s]][-1]:
                    gs = CHUNK_GROUPS[grp_chunk[s]]
                    dsl = slice(gs[0] * NG, (gs[-1] + 1) * NG)
                    nc.scalar.dma_start(out=q_rep[C8:64, dsl],
                                        in_=q_rep[0:C8, dsl])
                    nc.scalar.dma_start(out=q_rep[64:128, dsl],
                                        in_=q_rep[0:64, dsl])
                    nc.sync.dma_start(out=k_rep[0:C8, dsl],
                                      in_=k_rep[C8:64, dsl])
                    nc.sync.dma_start(out=k_rep[64:128, dsl],
                                      in_=k_rep[0:64, dsl])

            # residual chunks (first needed ~20us into attention)
            xr_t = []
            for h in range(2):
                tr = sing.tile([128, 16, C], bf16, tag=f"xr{h}", name=f"xr{h}")
                nc.sync.dma_start(
                    out=tr,
                    in_=xr_d[:].rearrange("(gs p) c -> p gs c", p=128)[
                        :, h * 16:(h + 1) * 16, :])
                xr_t.append(tr)

            # ---- attention, software-pipelined one group deep ----
            e_tiles = {}
            u_tiles = {}

            def issue_lt_exp(g, j):
                # rounds are emitted in pairs: even j uses row_grps 0/32,
                # odd j 64/96, so four K=32 logit matmuls sit back-to-back in
                # the PE queue and run concurrently in the four 32-row strips
                sl = slice(g * NG, (g + 1) * NG)
                plt = lt_ps.tile([128, EW, NG], f32, tag="plt", name="plt")
                for rg in range(EW if "lt" not in ablate else 1):
                    mc = EW * j + rg
                    msl = slice(mc * 128, (mc + 1) * 128)
                    row = ((j % 2) * 2 + rg) * C8
                    tp = (96, 0) if row == 96 else None
                    nc.tensor.matmul(plt[:, rg, :], k_rep[row:row + C8, msl],
                                     q_rep[row:row + C8, sl],
                                     start=True, stop=True, tile_position=tp)
                e_t = epool.tile([128, EW, NG], bf16, tag="e", name="e_t")
                fn = AF.Exp if "exp" not in ablate else AF.Copy
                nc.scalar.activation(e_t, plt, fn)
                e_tiles[(g, j)] = e_t

            def issue_av(g, j):
                uts = u_tiles[g]
                e_t = e_tiles.pop((g, j))
                if "av" in ablate:
                    if j == 0:
                        for sub in range(4):
                            nc.tensor.matmul(uts[sub],
                                             e_t[:, 0, sub * 128:(sub + 1) * 128],
                                             vt_sb[:, 0, 0:CP], start=True, stop=True)
                    return
                if j == RND - 1:
                    # last round sub-major: each sub's accumulation stops as
                    # early as possible so its epilogue overlaps remaining AV
                    for sub in range(4):
                        for rg in range(EW):
                            mc = EW * j + rg
                            nc.tensor.matmul(uts[sub],
                                             e_t[:, rg, sub * 128:(sub + 1) * 128],
                                             vt_sb[:, mc, 0:CP],
                                             start=False, stop=(rg == EW - 1))
                    return
                for rg in range(EW):
                    mc = EW * j + rg
                    first = (j == 0 and rg == 0)
                    for sub in range(4):
                        nc.tensor.matmul(uts[sub],
                                         e_t[:, rg, sub * 128:(sub + 1) * 128],
                                         vt_sb[:, mc, 0:CP],
                                         start=first, stop=False)

            def issue_epilogue(g):
                # DVE normalizes (gamma/Z); GPSIMD adds the bf16 residual.
                # Last group streams each sub's y out separately (short tail).
                uts = u_tiles.pop(g)
                y_view = y_d[:].rearrange("(gs p) c -> p gs c", p=128)
                y_g = ypool.tile([128, 4, C], bf16, tag="yg", name="y_g")
                for sub in range(4):
                    ut = uts[sub]
                    rinv = scal.tile([128, 1], f32, tag="rinv", name="rinv")
                    nc.vector.reciprocal(rinv, ut[:, C:CP])
                    gsc = scal.tile([128, 1], f32, tag="gsc", name="gsc")
                    nc.vector.tensor_scalar_mul(gsc, rinv, g128)
                    ysc = scal.tile([128, C], f32, tag="ysc", name="ysc")
                    nc.vector.tensor_scalar_mul(ysc, ut[:, 0:C], gsc)
                    nc.gpsimd.tensor_add(y_g[:, sub, :], ysc,
                                         xr_t[g // 4][:, (g % 4) * 4 + sub, :])
                    if g == G - 1:
                        nc.sync.dma_start(out=y_view[:, g * 4 + sub, :],
                                          in_=y_g[:, sub, :])
                if g < G - 1:
                    nc.sync.dma_start(out=y_view[:, g * 4:(g + 1) * 4, :],
                                      in_=y_g)

            for g in range(G + 1):
                if g < G:
                    u_tiles[g] = [u_ps.tile([128, CP], f32, tag=f"u{s}", name=f"u{s}")
                                  for s in range(4)]
                for jp in range(RND // 2):
                    if g < G:
                        issue_lt_exp(g, 2 * jp)
                        issue_lt_exp(g, 2 * jp + 1)
                    if g >= 1:
                        issue_av(g - 1, 2 * jp)
                        issue_av(g - 1, 2 * jp + 1)
                if g >= 1:
                    issue_epilogue(g - 1)

    nc.compile()
    return nc


def prepare_in_maps(inputs):
    """Host-side prep: bf16 casts, packed weights, residual fold."""
    import ml_dtypes
    bf = ml_dtypes.bfloat16
    x = np.asarray(inputs["x"], dtype=np.float32)
    wq = np.asarray(inputs["wq"], dtype=np.float32)
    bq = np.asarray(inputs["bq"], dtype=np.float32)
    wk = np.asarray(inputs["wk"], dtype=np.float32)
    wv = np.asarray(inputs["wv"], dtype=np.float32)
    bv = np.asarray(inputs["bv"], dtype=np.float32)
    gamma = np.asarray(inputs["gamma"], dtype=np.float32)

    xr = np.ascontiguousarray(x.reshape(B, C, N))
    x_h = xr.astype(bf)
    # residual in [N, C] layout with gamma*bv folded in (bf16: |err| ~0.4%)
    xres = np.ascontiguousarray(
        xr.transpose(0, 2, 1) + gamma[0] * bv[None, None, :]).astype(bf)

    # pack all bf16 weights partition-major into [128, 640]:
    #   cols 0:128 = 2 cc-slots of wqkT ([wq.T wk.T]), cols 128:640 = wvT
    wqkT = np.concatenate([wq.T, wk.T], axis=1).astype(bf)   # [(cc p), 64]
    wqk_pm = wqkT.reshape(2, 128, 64).transpose(1, 0, 2).reshape(128, 128)
    wvT = wv.T.astype(bf)                                    # [(cc p), C]
    wv_pm = wvT.reshape(2, 128, C).transpose(1, 0, 2).reshape(128, 512)
    wpack = np.ascontiguousarray(np.concatenate([wqk_pm, wv_pm], axis=1))
    # bqg: col 0 = gamma replicated, col 1 rows 0:32 = bq
    bqg = np.zeros((128, 2), dtype=np.float32)
    bqg[:, 0] = gamma[0]
    bqg[0:C8, 1] = bq

    shared = {"wpack": wpack, "bqg": bqg}
    return [dict(shared,
                 x_h=np.ascontiguousarray(x_h[i]),
                 x_res=xres[i]) for i in range(B)]


_nc_cache = None


def kernel(**inputs) -> np.ndarray:
    global _nc_cache
    if _nc_cache is None:
        _nc_cache = build_program()
    nc = _nc_cache

    in_maps = prepare_in_maps(inputs)
    res = run_bass_kernel_spmd(nc, in_maps, core_ids=list(range(B)))
    # y comes back [N, C] bf16 per core; transpose to [C, N] on host
    y = np.stack([res.results[i]["y"].astype(np.float32).T for i in range(B)],
                 axis=0)
    return np.ascontiguousarray(y.reshape(B, C, 64, 64))


if __name__ == "__main__":
    rng = np.random.default_rng(0)
    ins = {
        "x": rng.standard_normal((B, C, 64, 64), dtype=np.float32),
        "wq": rng.standard_normal((C8, C), dtype=np.float32) / 16,
        "bq": rng.standard_normal((C8,), dtype=np.float32) * 0.01,
        "wk": rng.standard_normal((C8, C), dtype=np.float32) / 16,
        "bk": rng.standard_normal((C8,), dtype=np.float32) * 0.01,
        "wv": rng.standard_normal((C, C), dtype=np.float32) / 16,
        "bv": rng.standard_normal((C,), dtype=np.float32) * 0.01,
        "gamma": rng.standard_normal((1,), dtype=np.float32) * 0.1,
    }
    out = kernel(**ins)
    print("kernel output", out.shape, out.dtype)
